# revision 8
# baseline (speedup 1.0000x reference)
"""DRR (Siddon ray-trace) Trainium2 kernel.

Geometry insight: for this problem the detector plane is perpendicular to a
volume axis (the "driving" axis, x after permutation). Every ray crosses
x-plane k at the SAME parameter alpha A_k, and within one x-slab the sample's
y-coordinate depends only on the detector column j while z depends only on the
row i. The Siddon sort+gather therefore reduces, per slab, to a separable
resampling of one 256x256 slice with per-slab host-computable indices:

  contrib[i,j] = w1*V[Yl(j),Zl(i)] + wmy*V[Yr(j),Zl(i)]
               + wmz*V[Yl(j),Zr(i)] + w3*V[Yr(j),Zr(i)]

with piece boundaries built from the (clamped) in-slab y-crossing ay(j) and
z-crossing az(i). The four gathered images are two chained one-hot matmuls on
the PE (z-select then y-select); the weights are ~20 elementwise DVE ops.
Rays nearly parallel to a y/z plane (where the separable approximation can
flip a voxel index) are recomputed exactly on host and patched (2 of 512
detector lines here).

Sharding: slabs are sharded across the 8 cores (core c owns slabs
[32c, 32c+32)); each core emits a partial 256x256 image, summed on host.
"""

import os
import numpy as np

import concourse.bass as bass
import concourse.bacc as bacc
import concourse.mybir as mybir
import concourse.tile as tile
from concourse.bass_utils import run_bass_kernel_spmd

F32 = mybir.dt.float32
N = 256          # volume side / detector side
NCORES = 8
SLABS_PER_CORE = N // NCORES   # 32
EPS = 1e-8

LAST_EXEC_NS = None  # set when BASS_TRACE=1


def _host_geometry(volume, spacing, sdr, rotations, translations):
    """Mimic reference's float32 geometry, then do crossing math in f64."""
    f32 = np.float32
    th, ph, ga = [f32(x) for x in np.asarray(rotations, dtype=f32)]
    ct, st = np.cos(th, dtype=f32), np.sin(th, dtype=f32)
    cp, sp = np.cos(ph, dtype=f32), np.sin(ph, dtype=f32)
    cg, sg = np.cos(ga, dtype=f32), np.sin(ga, dtype=f32)
    Rz = np.array([[ct, -st, 0], [st, ct, 0], [0, 0, 1]], dtype=f32)
    Ry = np.array([[cp, 0, sp], [0, 1, 0], [-sp, 0, cp]], dtype=f32)
    Rx = np.array([[1, 0, 0], [0, cg, -sg], [0, sg, cg]], dtype=f32)
    R = (Rz @ Ry @ Rx).astype(f32)
    sdr_f = f32(np.asarray(sdr, dtype=f32).reshape(-1)[0])
    tr = np.asarray(translations, dtype=f32)
    source = (sdr_f * R[:, 0] + tr).astype(f32)          # (3,)
    center = (-sdr_f * R[:, 0] + tr).astype(f32)
    u, v = R[:, 1].astype(f32), R[:, 2].astype(f32)
    t = ((np.arange(-(N // 2), N // 2, dtype=f32) + 1.0) * 2.0).astype(f32)  # rows i
    s = ((np.arange(-(N // 2), N // 2, dtype=f32) + 1.0) * 2.0).astype(f32)  # cols j

    # full targets for exact raylength  (i over t, j over s)
    targets = (t[:, None, None] * u[None, None, :]
               + s[None, :, None] * v[None, None, :]
               + center[None, None, :]).astype(f32)       # (N, N, 3)
    sdd_full = (targets - source[None, None, :] + f32(EPS)).astype(f32)
    raylength = np.sqrt((sdd_full.astype(np.float64) ** 2).sum(-1))  # (N, N)

    # axes: driving d (detector normal), axis_j moved by s (v), axis_i by t (u)
    d = int(np.argmax(np.abs(R[:, 0])))
    axis_j = int(np.argmax(np.abs(v)))
    axis_i = int(np.argmax(np.abs(u)))
    assert {d, axis_j, axis_i} == {0, 1, 2}, (d, axis_j, axis_i)

    sp3 = np.asarray(spacing, dtype=np.float64)
    src = source.astype(np.float64)
    cen = center.astype(np.float64)

    # separable ray direction components (f64 from f32 pieces, matching ref +EPS)
    ddx = cen[d] - src[d] + EPS                                   # const
    dy = (cen[axis_j] + s.astype(np.float64) * np.float64(v[axis_j])
          - src[axis_j] + EPS)                                    # (N,) per j
    dz = (cen[axis_i] + t.astype(np.float64) * np.float64(u[axis_i])
          - src[axis_i] + EPS)                                    # (N,) per i
    if ddx < 0:
        raise NotImplementedError("flip driving axis not needed for this input")
    dy = np.where(np.abs(dy) < 1e-300, 1e-300, dy)
    dz = np.where(np.abs(dz) < 1e-300, 1e-300, dz)

    # normalized (voxel-unit) coords
    sxh, dxh = src[d] / sp3[d], ddx / sp3[d]
    syh, dyh = src[axis_j] / sp3[axis_j], dy / sp3[axis_j]
    szh, dzh = src[axis_i] / sp3[axis_i], dz / sp3[axis_i]

    A = (np.arange(N + 1, dtype=np.float64) - sxh) / dxh          # (257,)
    dA = np.diff(A).max()
    assert np.abs(dyh).max() * dA < 1.0 and np.abs(dzh).max() * dA < 1.0

    def axis_cross(sh, dh):
        """Per (k, col): clamped crossing alpha + voxel idx before/after."""
        Ak, Ak1 = A[:N, None], A[1:, None]
        y0 = sh + Ak * dh[None, :]
        y1 = sh + Ak1 * dh[None, :]
        f0, f1 = np.floor(y0), np.floor(y1)
        p = np.maximum(f0, f1)
        ac = np.where(f0 != f1, (p - sh) / dh[None, :], Ak1)
        ac = np.clip(ac, Ak, Ak1)
        il = np.clip(np.floor(sh + dh[None, :] * (Ak + ac) * 0.5), 0, N - 1)
        ir = np.clip(np.floor(sh + dh[None, :] * (ac + Ak1) * 0.5), 0, N - 1)
        a0, a1 = (0.0 - sh) / dh, (np.float64(N) - sh) / dh
        amax_ax = np.maximum(a0, a1)
        amin_ax = np.minimum(a0, a1)
        return ac, il.astype(np.int64), ir.astype(np.int64), amax_ax, amin_ax

    ay, Yl, Yr, aymax, aymin = axis_cross(syh, dyh)   # (256k, 256j)
    az, Zl, Zr, azmax, azmin = axis_cross(szh, dzh)   # (256k, 256i)
    assert aymin.max() <= A[0] + 1e-9 and azmin.max() <= A[0] + 1e-9, \
        "entry clipping not on driving axis; unsupported geometry"

    perm = (d, axis_j, axis_i)
    vol_p = np.ascontiguousarray(np.transpose(np.asarray(volume, dtype=f32), perm))
    return dict(vol_p=vol_p, A=A, ay=ay, az=az, Yl=Yl, Yr=Yr, Zl=Zl, Zr=Zr,
                aymax=aymax, azmax=azmax, raylength=raylength,
                dyh=dyh, dzh=dzh, source=source, targets=targets)


def _exact_drr_f32(volume, spacing, source, targets):
    """Reference Siddon replicated in float32 numpy for a few rays (R, 3)."""
    f32 = np.float32
    vol = np.asarray(volume, dtype=f32)
    sp = np.asarray(spacing, dtype=f32)
    src = np.asarray(source, dtype=f32)
    sdd = (targets - src + f32(EPS)).astype(f32)                 # (R, 3)
    dims = np.asarray([N + 1, N + 1, N + 1], dtype=f32)
    a0 = (f32(0.0) * sp - src) / sdd
    a1 = ((dims - f32(1.0)) * sp - src) / sdd
    amin = np.max(np.minimum(a0, a1), axis=-1, keepdims=True)
    amax = np.min(np.maximum(a0, a1), axis=-1, keepdims=True)
    planes = np.arange(N + 1, dtype=f32)
    alphas = np.concatenate(
        [(planes * sp[c] - src[c]) / sdd[:, c:c + 1] for c in range(3)],
        axis=-1).astype(f32)                                     # (R, 3N+3)
    good = (alphas >= amin) & (alphas <= amax)
    alphas = np.where(good, alphas, amax).astype(f32)
    alphas = np.sort(alphas, axis=-1)
    amid = (f32(0.5) * (alphas[:, :-1] + alphas[:, 1:])).astype(f32)
    step = np.diff(alphas, axis=-1)
    pts = (src + amid[..., None] * sdd[:, None, :]).astype(f32)
    idx = np.clip(np.trunc(pts / sp).astype(np.int32), 0, N - 1)
    vox = vol[idx[..., 0], idx[..., 1], idx[..., 2]]
    drr = (vox * step).sum(axis=-1, dtype=f32)
    ray = np.sqrt((sdd.astype(f32) ** 2).sum(-1, dtype=f32)).astype(f32)
    return drr * ray


def _patch_degenerate(drr_img, g, volume, spacing):
    """Overwrite rays nearly parallel to a y/z plane with exact f32 Siddon."""
    bad_j = np.where(np.abs(g["dyh"]) < 1e-2)[0]
    bad_i = np.where(np.abs(g["dzh"]) < 1e-2)[0]
    if len(bad_j) == 0 and len(bad_i) == 0:
        return drr_img
    ii, jj = [], []
    for j in bad_j:
        ii.append(np.arange(N)); jj.append(np.full(N, j))
    for i in bad_i:
        ii.append(np.full(N, i)); jj.append(np.arange(N))
    ii = np.concatenate(ii); jj = np.concatenate(jj)
    tg = g["targets"][ii, jj]                                    # (R, 3)
    drr_img[ii, jj] = _exact_drr_f32(volume, spacing, g["source"], tg)
    return drr_img


def _build_program():
    """One program for all 8 cores; per-core data differs only in inputs.

    Per-slab packed inputs (single DMA each, to stay within the per-
    instruction sync-wait limit):
      mm1[s]  : [128, 1536] = vt0|vt1|szl0|szl1|szr0|szr1  (z on partitions)
      mm2[s]  : [128, 1024] = syl0|syl1|syr0|syr1          (y on partitions)
      wvec[s] : [128, 260]  = azrep(256) | ay_jt0 | ay_jt1 | A_k | A_k1
      amaxi   : [2, 128, 256] clamped per-ray alpha_max image (per j-tile)
    """
    nc = bacc.Bacc("TRN2")
    S = SLABS_PER_CORE
    mm1_d = nc.declare_dram_parameter("mm1", [S, 128, 1536], F32, False)
    mm2_d = nc.declare_dram_parameter("mm2", [S, 128, 1024], F32, False)
    wv_d = nc.declare_dram_parameter("wvec", [S, 128, 260], F32, False)
    amx_d = nc.declare_dram_parameter("amaxi", [2, 128, N], F32, False)
    out_d = nc.declare_dram_parameter("out", [N, N], F32, True)   # [j, i]

    ts = bass.ts
    op = mybir.AluOpType
    with tile.TileContext(nc) as tc:
        with (
            tc.tile_pool(name="const", bufs=1) as cst,
            tc.tile_pool(name="inp", bufs=3) as inp,
            tc.tile_pool(name="tsb", bufs=2) as tsbp,
            tc.tile_pool(name="tmp", bufs=3) as tmp,
            tc.tile_pool(name="psT", bufs=4, space="PSUM") as psT,
            tc.tile_pool(name="psG", bufs=4, space="PSUM") as psG,
        ):
            acc = []
            amax = []
            for jt in range(2):
                a = cst.tile([128, N], F32, tag=f"acc{jt}")
                nc.vector.memset(a[:], 0.0)
                acc.append(a)
                am = cst.tile([128, N], F32, tag=f"amax{jt}")
                nc.gpsimd.dma_start(out=am[:], in_=amx_d[jt])
                amax.append(am)

            for s in range(S):
                mm1 = inp.tile([128, 1536], F32, tag="mm1")
                nc.gpsimd.dma_start(out=mm1[:], in_=mm1_d[s])
                mm2 = inp.tile([128, 1024], F32, tag="mm2")
                nc.gpsimd.dma_start(out=mm2[:], in_=mm2_d[s])
                wv = inp.tile([128, 260], F32, tag="wv")
                nc.gpsimd.dma_start(out=wv[:], in_=wv_d[s])

                azr = wv[:, 0:N]
                ayp = [wv[:, N + jt:N + jt + 1] for jt in range(2)]
                ak, ak1 = wv[:, 258:259], wv[:, 259:260]

                # T_a[y, i] = V[y, Za(i)]  (z-select); a=0 -> Zl, a=1 -> Zr
                T = {}
                for a in range(2):
                    for yb in range(2):
                        tp = psT.tile([128, N], F32, tag="T")
                        for zc in range(2):
                            nc.tensor.matmul(
                                tp[:],
                                lhsT=mm1[:, zc * 256 + yb * 128:
                                         zc * 256 + (yb + 1) * 128],
                                rhs=mm1[:, 512 + a * 512 + zc * 256:
                                        512 + a * 512 + (zc + 1) * 256],
                                start=(zc == 0), stop=(zc == 1))
                        tsb = tsbp.tile([128, N], F32, tag=f"T{a}{yb}")
                        nc.vector.tensor_copy(tsb[:], tp[:])
                        T[(a, yb)] = tsb

                def bc(col):
                    # [128,1] AP -> [128,N] stride-0 free-dim broadcast
                    b, _ = bass.broadcast_tensor_aps(col, azr)
                    return b

                for jt in range(2):
                    ayb = bc(ayp[jt])
                    e = tmp.tile([128, N], F32, tag="e")
                    nc.vector.tensor_tensor(e[:], amax[jt][:], bc(ak1), op.min)
                    nc.vector.tensor_tensor(e[:], e[:], bc(ak), op.max)
                    c1 = tmp.tile([128, N], F32, tag="c1")
                    nc.vector.tensor_tensor(c1[:], azr, ayb, op.min)
                    nc.vector.tensor_tensor(c1[:], c1[:], e[:], op.min)
                    c2 = tmp.tile([128, N], F32, tag="c2")
                    nc.vector.tensor_tensor(c2[:], azr, ayb, op.max)
                    nc.vector.tensor_tensor(c2[:], c2[:], e[:], op.min)
                    w1 = tmp.tile([128, N], F32, tag="w1")
                    nc.vector.tensor_tensor(w1[:], c1[:], bc(ak), op.subtract)
                    wm = tmp.tile([128, N], F32, tag="wm")
                    nc.vector.tensor_tensor(wm[:], c2[:], c1[:], op.subtract)
                    w3 = tmp.tile([128, N], F32, tag="w3")
                    nc.vector.tensor_tensor(w3[:], e[:], c2[:], op.subtract)
                    msk = tmp.tile([128, N], F32, tag="msk")
                    nc.vector.tensor_tensor(msk[:], azr, ayb, op.is_gt)
                    wmy = tmp.tile([128, N], F32, tag="wmy")
                    nc.vector.tensor_tensor(wmy[:], wm[:], msk[:], op.mult)
                    wmz = tmp.tile([128, N], F32, tag="wmz")
                    nc.vector.tensor_tensor(wmz[:], wm[:], wmy[:], op.subtract)

                    # G_{a,b}[j,i] = V[Yb(j), Za(i)]; weights: (a,b) ->
                    # (Zl,Yl):w1  (Zl,Yr):wmy  (Zr,Yl):wmz  (Zr,Yr):w3
                    for a, b, w in ((0, 0, w1), (0, 1, wmy),
                                    (1, 0, wmz), (1, 1, w3)):
                        g = psG.tile([128, N], F32, tag="G")
                        for yc in range(2):
                            nc.tensor.matmul(
                                g[:],
                                lhsT=mm2[:, b * 512 + yc * 256 + jt * 128:
                                         b * 512 + yc * 256 + (jt + 1) * 128],
                                rhs=T[(a, yc)][:],
                                start=(yc == 0), stop=(yc == 1))
                        p = tmp.tile([128, N], F32, tag="p")
                        nc.vector.tensor_tensor(p[:], g[:], w[:], op.mult)
                        nc.vector.tensor_tensor(acc[jt][:], acc[jt][:], p[:],
                                                op.add)

            for jt in range(2):
                nc.gpsimd.dma_start(out=out_d[ts(jt, 128), :], in_=acc[jt][:])
    return nc


def _onehot_cols(idx):
    """idx: (K, M) int -> (K, N, M) f32 one-hot over middle axis."""
    K, M = idx.shape
    oh = np.zeros((K, N, M), dtype=np.float32)
    kk, mm = np.meshgrid(np.arange(K), np.arange(M), indexing="ij")
    oh[kk, idx, mm] = 1.0
    return oh


def _pack_inputs(g):
    """Build the per-core input maps (see _build_program docstring)."""
    f32 = np.float32
    A, ay, az = g["A"], g["ay"], g["az"]
    S = SLABS_PER_CORE

    syl = _onehot_cols(g["Yl"])   # [k, y, j]
    syr = _onehot_cols(g["Yr"])
    szl = _onehot_cols(g["Zl"])   # [k, z, i]
    szr = _onehot_cols(g["Zr"])

    amaxi = np.minimum(np.minimum(g["azmax"][None, :], A[N]),
                       g["aymax"][:, None]).astype(f32)          # [j, i]
    amaxi = np.ascontiguousarray(amaxi.reshape(2, 128, N))

    in_maps = []
    for c in range(NCORES):
        sl = slice(c * S, (c + 1) * S)
        ks = np.arange(c * S, (c + 1) * S)
        vt = g["vol_p"][sl].transpose(0, 2, 1)                   # [S, z, y]
        mm1 = np.empty((S, 128, 1536), dtype=f32)
        mm2 = np.empty((S, 128, 1024), dtype=f32)
        for zc in range(2):
            zs = slice(zc * 128, (zc + 1) * 128)
            mm1[:, :, zc * 256:(zc + 1) * 256] = vt[:, zs, :]
            mm1[:, :, 512 + zc * 256:512 + (zc + 1) * 256] = szl[sl][:, zs, :]
            mm1[:, :, 1024 + zc * 256:1024 + (zc + 1) * 256] = szr[sl][:, zs, :]
            mm2[:, :, zc * 256:(zc + 1) * 256] = syl[sl][:, zs, :]
            mm2[:, :, 512 + zc * 256:512 + (zc + 1) * 256] = syr[sl][:, zs, :]
        wvec = np.empty((S, 128, 260), dtype=f32)
        wvec[:, :, 0:N] = az[sl].astype(f32)[:, None, :]
        wvec[:, :, N:N + 2] = \
            ay[sl].astype(f32).reshape(S, 2, 128).transpose(0, 2, 1)
        wvec[:, :, 258] = A[ks].astype(f32)[:, None]
        wvec[:, :, 259] = A[ks + 1].astype(f32)[:, None]
        in_maps.append({"mm1": mm1, "mm2": mm2, "wvec": wvec, "amaxi": amaxi})
    return in_maps


def kernel(volume, spacing, sdr, rotations, translations):
    global LAST_EXEC_NS
    g = _host_geometry(volume, spacing, sdr, rotations, translations)
    in_maps = _pack_inputs(g)
    nc = _build_program()
    if not nc.is_finalized():
        nc.finalize()

    kw = {}
    if os.environ.get("BASS_TRACE", "0") == "1":
        tmpdir = "/root/problem/trace_out"
        os.makedirs(tmpdir, exist_ok=True)
        kw = dict(trace=True, tmpdir=tmpdir)
    res = run_bass_kernel_spmd(nc, in_maps, list(range(NCORES)), **kw)
    LAST_EXEC_NS = res.exec_time_ns

    total = np.zeros((N, N), dtype=np.float64)
    for c in range(NCORES):
        total += res.results[c]["out"].astype(np.float64)
    drr = total.T * g["raylength"]                          # [i, j]
    drr = _patch_degenerate(drr, g, volume, spacing)
    return drr.reshape(-1).astype(np.float32)
